# revision 28
# baseline (speedup 1.0000x reference)
"""Trainium2 Bass kernel for nn_MultiHeadAttention_36112085025201.

Multi-head attention, B=2, S=4096, D=512, H=8 heads, Dh=64.
Sharding: 8 cores = 2 (batch) x 4 (head-pairs). Each core computes its
batch's attention for 2 heads plus that head-slice's contribution to the
output projection; the host sums the 4 partial projections per batch.

Per-core algorithm (all matmuls bf16, accumulation fp32 in PSUM):
  - inputs arrive pre-transposed/sliced: xT [D,S] bf16, wq/wk/wv [D,128],
    wo [128,D], biases.
  - qT,kT [128,S] = w.T @ x.T (transposed orientation, per-partition bias)
  - v [S,128] (normal orientation, no bias: folded into bvwo row)
  - streaming attention per (head, 512-wide query block):
      for each 128-row key chunk: scoresT = k_chunk @ qT (PSUM) ->
      exp on ACT (scale=1/8, no max-subtraction: scores provably small) ->
      PV matmul accumulates [v | ones].T @ expT, giving unnormalized
      attention output rows 0..63 and the softmax denominator in row 64.
  - normalize: DVE approx reciprocal of the denominator row + GPSIMD
    partition broadcast + DVE multiply (deferred out-projection keeps the
    PE queue busy across block boundaries)
  - out projection per 128-row tile; the fused (bv@wo + bo) bias row is
    added during PSUM eviction on the DVE.
"""

import numpy as np
from contextlib import ExitStack

import ml_dtypes
import concourse.tile as tile
from concourse import bacc, mybir
from concourse.bass_utils import run_bass_kernel_spmd

# Problem constants (hardcoded per harness contract).
B, S, D = 2, 4096, 512
H, Dh = 8, 64
SCALE = Dh ** -0.5
N_CORES = 8
HL = 2                 # heads per core
CW = HL * Dh           # 128 local head columns per core
NK = D // 128          # 4 contraction chunks for projections
NSQ = S // 512         # 8 query blocks
NST = S // 128         # 32 key chunks (also 128-row output tiles)
VW = Dh + 1            # v width incl. ones column

BF16 = mybir.dt.bfloat16
F32 = mybir.dt.float32
EXP = mybir.ActivationFunctionType.Exp


def _build_body(ctx: ExitStack, tc: "tile.TileContext", io: dict, dbg: dict | None = None):
    nc = tc.nc
    xT, wq, wk, wv, wo = io["xT"], io["wq"], io["wk"], io["wv"], io["wo"]
    bq, bk, bvwo, out = io["bq"], io["bk"], io["bvwo"], io["out"]

    const = ctx.enter_context(tc.tile_pool(name="const", bufs=1))
    persist = ctx.enter_context(tc.tile_pool(name="persist", bufs=1))

    # Persistent SBUF arrays.
    xT_sb = [persist.tile([128, S], BF16, tag=f"xT{k}", name=f"xT{k}") for k in range(NK)]
    qT_sb = persist.tile([128, S], BF16, tag="qT")
    kT_sb = persist.tile([128, S], BF16, tag="kT")
    vext = [persist.tile([128, VW * NST], BF16, tag=f"vext{h}", name=f"vext{h}") for h in range(HL)]
    onormT = persist.tile([128, S], BF16, tag="onormT")

    wq_sb = [const.tile([128, CW], BF16, tag=f"wq{k}", name=f"wq{k}") for k in range(NK)]
    wk_sb = [const.tile([128, CW], BF16, tag=f"wk{k}", name=f"wk{k}") for k in range(NK)]
    wv_sb = [const.tile([128, CW], BF16, tag=f"wv{k}", name=f"wv{k}") for k in range(NK)]
    wo_sb = const.tile([128, D], BF16, tag="wo")
    bq_sb = const.tile([CW, 1], F32, tag="bq")
    bk_sb = const.tile([CW, 1], F32, tag="bk")
    bvwo_sb = const.tile([1, D], F32, tag="bvwo")
    bvwo_bc = const.tile([128, D], F32, tag="bvwo_bc")

    # Input DMAs. Weights first (small, gate everything), then xT in
    # column chunks ordered the way the projections consume them, so the
    # first projection matmuls start after ~1MB instead of ~4MB of DMA.
    for k in range(NK):
        nc.sync.dma_start(wq_sb[k][:], wq[128 * k:128 * (k + 1), :])
        nc.sync.dma_start(wk_sb[k][:], wk[128 * k:128 * (k + 1), :])
        nc.sync.dma_start(wv_sb[k][:], wv[128 * k:128 * (k + 1), :])
    for jp in range(NSQ // 2):
        for k in range(NK):
            nc.sync.dma_start(xT_sb[k][:, 1024 * jp:1024 * (jp + 1)],
                              xT[128 * k:128 * (k + 1), 1024 * jp:1024 * (jp + 1)])
    nc.sync.dma_start(wo_sb[:], wo[:, :])
    nc.sync.dma_start(bq_sb[:], bq[:, :])
    nc.sync.dma_start(bk_sb[:], bk[:, :])
    nc.sync.dma_start(bvwo_sb[:], bvwo[:, :])

    # PSUM pools (8 banks total on TRN2): pmm 2x[128,1024] = 4 banks,
    # pacc 4x[65,512] = 4 banks.
    pmm = ctx.enter_context(tc.tile_pool(name="pmm", bufs=2, space="PSUM"))
    pacc = ctx.enter_context(tc.tile_pool(name="pacc", bufs=1, space="PSUM"))

    # One-time: broadcast the fused bias row across partitions.
    nc.gpsimd.partition_broadcast(bvwo_bc[:], bvwo_sb[:])

    expp = ctx.enter_context(tc.tile_pool(name="expp", bufs=3))
    rp = ctx.enter_context(tc.tile_pool(name="rp", bufs=4))
    outp = ctx.enter_context(tc.tile_pool(name="outp", bufs=3))

    # Phase A/B: projections. Emission order: k first (phase C's t-loop
    # needs all of kT), then v, then q (phase C consumes q blocks in order).
    # Two 512-blocks share one [128,1024] psum tile so each LDWEIGHTS serves
    # two matmuls and the DVE eviction is amortized.
    def qk_proj(w_sb, b_sb, dst):
        for jp in range(NSQ // 2):
            ps = pmm.tile([128, 1024], F32, tag="mm")
            for k in range(NK):
                for jj in range(2):
                    nc.tensor.matmul(ps[:, 512 * jj:512 * (jj + 1)], w_sb[k][:],
                                     xT_sb[k][:, 1024 * jp + 512 * jj:1024 * jp + 512 * (jj + 1)],
                                     start=(k == 0), stop=(k == NK - 1))
            nc.vector.tensor_scalar_add(dst[:, 1024 * jp:1024 * (jp + 1)], ps[:], b_sb[:])

    qk_proj(wk_sb, bk_sb, kT_sb)

    # v projection in normal orientation [s, c], split per head into vext
    # tiles [128, 65] with a trailing ones column (memset first).
    for h in range(HL):
        nc.vector.memset(vext[h][:], 1.0)
    for tp in range(NST // 2):
        ps = pmm.tile([128, 1024], F32, tag="mm")
        for tt in range(2):
            t = 2 * tp + tt
            for k in range(NK):
                nc.tensor.matmul(ps[:, 512 * tt:512 * tt + CW],
                                 xT_sb[k][:, 128 * t:128 * (t + 1)], wv_sb[k][:],
                                 start=(k == 0), stop=(k == NK - 1))
        for tt in range(2):
            t = 2 * tp + tt
            for h in range(HL):
                nc.vector.tensor_copy(vext[h][:, VW * t:VW * t + Dh],
                                      ps[:, 512 * tt + Dh * h:512 * tt + Dh * (h + 1)])

    qk_proj(wq_sb, bq_sb, qT_sb)

    # Phase C: streaming attention + interleaved output projection.
    # Two query blocks (one [128,1024] scores psum per head) per key-chunk
    # iteration: each kT/v LDWEIGHTS serves two matmuls, same-weight matmuls
    # run back-to-back, and the h0/h1 groups occupy disjoint PE row groups
    # so they co-execute. exp runs once per head over [128,1024].
    def out_proj_prev(jp_):
        for st in range(8):
            sq0 = 1024 * jp_ + 128 * st
            pf = pmm.tile([128, 1024], F32, tag="mm", name="pf")
            nc.tensor.matmul(pf[:, 0:512], onormT[:, sq0:sq0 + 128], wo_sb[:],
                             start=True, stop=True)
            ob = outp.tile([128, 512], F32, tag="ob")
            nc.vector.tensor_add(ob[:], pf[:, 0:512], bvwo_bc[:])
            nc.sync.dma_start(out[sq0:sq0 + 128, :], ob[:])

    for jp in range(NSQ // 2):
        j0 = 2 * jp
        po = {(h, jj): pacc.tile([VW, 512], F32, tag=f"acc{h}{jj}", name=f"po{h}{jj}")
              for h in range(HL) for jj in range(2)}
        # Software-pipelined: PV for key-chunk t-1 is emitted after the
        # scores+exp of chunk t, so exp latency hides behind the next
        # chunk's score matmuls instead of stalling the PE queue.
        def emit_pv(e_prev, t_prev):
            for h in range(HL):
                for jj in range(2):
                    nc.tensor.matmul(po[(h, jj)][:],
                                     vext[h][:, VW * t_prev:VW * (t_prev + 1)],
                                     e_prev[h][:, 512 * jj:512 * (jj + 1)],
                                     start=(t_prev == 0), stop=(t_prev == NST - 1))

        e_prev = None
        for t in range(NST):
            s = {}
            for h in range(HL):
                s[h] = pmm.tile([128, 1024], F32, tag="mm", name=f"s{h}")
                for jj in range(2):
                    nc.tensor.matmul(s[h][:, 512 * jj:512 * (jj + 1)],
                                     kT_sb[Dh * h:Dh * (h + 1), 128 * t:128 * (t + 1)],
                                     qT_sb[Dh * h:Dh * (h + 1),
                                           512 * (j0 + jj):512 * (j0 + jj + 1)],
                                     start=True, stop=True)
            e_cur = {}
            for h in range(HL):
                e_cur[h] = expp.tile([128, 1024], BF16, tag="e", bufs=4, name=f"e{h}")
                nc.scalar.activation(e_cur[h][:], s[h][:], EXP, scale=float(SCALE))
            if e_prev is not None:
                emit_pv(e_prev, t - 1)
            e_prev = e_cur
        emit_pv(e_prev, NST - 1)
        if jp > 0:
            out_proj_prev(jp - 1)
        for h in range(HL):
            for jj in range(2):
                j = j0 + jj
                # NB: custom-DVE ucode ops (reciprocal_approx_*) mis-execute
                # at base partition != 0 on HW, and PSUM partition offsets
                # must be 32-aligned; copy the denominator row (partition 64)
                # to partition 0 first.
                r0 = rp.tile([1, 512], F32, tag="r0")
                nc.vector.tensor_copy(r0[:], po[(h, jj)][Dh:VW, :])
                r = rp.tile([1, 512], F32, tag="r")
                nc.vector.reciprocal_approx_fast(r[:], r0[:])
                rb = rp.tile([Dh, 512], F32, tag="rb")
                nc.gpsimd.partition_broadcast(rb[:], r[:])
                nc.vector.tensor_mul(onormT[Dh * h:Dh * (h + 1), 512 * j:512 * (j + 1)],
                                     po[(h, jj)][0:Dh, :], rb[:])
        # Output projection is deferred by one block pair (see
        # out_proj_prev call above) so the PE queue never idles on the
        # normalize chain at block boundaries; the last pair runs here.
        if jp == NSQ // 2 - 1:
            out_proj_prev(jp)

    if dbg:
        for name, sb in (("qT", qT_sb), ("kT", kT_sb), ("onormT", onormT),
                         ("vext0", vext[0]), ("vext1", vext[1])):
            if name in dbg:
                nc.sync.dma_start(dbg[name][:, :], sb[:])


def build_nc():
    nc = bacc.Bacc("TRN2", target_bir_lowering=False, debug=False,
                   enable_asserts=False, num_devices=N_CORES)
    io = {
        "xT": nc.dram_tensor("xT", [D, S], BF16, kind="ExternalInput").ap(),
        "wq": nc.dram_tensor("wq", [D, CW], BF16, kind="ExternalInput").ap(),
        "wk": nc.dram_tensor("wk", [D, CW], BF16, kind="ExternalInput").ap(),
        "wv": nc.dram_tensor("wv", [D, CW], BF16, kind="ExternalInput").ap(),
        "wo": nc.dram_tensor("wo", [CW, D], BF16, kind="ExternalInput").ap(),
        "bq": nc.dram_tensor("bq", [CW, 1], F32, kind="ExternalInput").ap(),
        "bk": nc.dram_tensor("bk", [CW, 1], F32, kind="ExternalInput").ap(),
        "bvwo": nc.dram_tensor("bvwo", [1, D], F32, kind="ExternalInput").ap(),
        "out": nc.dram_tensor("out", [S, D], F32, kind="ExternalOutput").ap(),
    }
    with tile.TileContext(nc) as tc, ExitStack() as ctx:
        _build_body(ctx, tc, io)
    nc.compile()
    return nc


def make_in_maps(x, wq, bq, wk, bk, wv, bv, wo, bo):
    """Shard the full inputs across the 8 cores (host-side marshalling)."""
    bf16 = ml_dtypes.bfloat16
    in_maps = []
    for c in range(N_CORES):
        b, hp = divmod(c, 4)
        cs = slice(CW * hp, CW * (hp + 1))
        xT = np.ascontiguousarray(x[b].T).astype(bf16)
        # Fused output-side bias row: bv@wo for this head slice, plus the full
        # bo on exactly one shard per batch (host sums the 4 partials).
        bvwo = (bv[cs].astype(np.float64) @ wo[cs, :].astype(np.float64))
        if hp == 0:
            bvwo = bvwo + bo.astype(np.float64)
        in_maps.append({
            "xT": xT,
            "wq": np.ascontiguousarray(wq[:, cs]).astype(bf16),
            "wk": np.ascontiguousarray(wk[:, cs]).astype(bf16),
            "wv": np.ascontiguousarray(wv[:, cs]).astype(bf16),
            "wo": np.ascontiguousarray(wo[cs, :]).astype(bf16),
            "bq": np.ascontiguousarray(bq[cs].reshape(CW, 1)).astype(np.float32),
            "bk": np.ascontiguousarray(bk[cs].reshape(CW, 1)).astype(np.float32),
            "bvwo": bvwo.reshape(1, D).astype(np.float32),
        })
    return in_maps


_CACHE = {}


def _get_nc():
    if "nc" not in _CACHE:
        _CACHE["nc"] = build_nc()
    return _CACHE["nc"]


def run_sharded(nc, in_maps, **kwargs):
    return run_bass_kernel_spmd(nc, in_maps, core_ids=list(range(N_CORES)), **kwargs)


def gather(results):
    out = np.zeros((B, S, D), np.float32)
    for c in range(N_CORES):
        out[c // 4] += results[c]["out"]
    return out


def kernel(x, wq, bq, wk, bk, wv, bv, wo, bo):
    x, wq, bq, wk, bk, wv, bv, wo, bo = (
        np.asarray(a, np.float32) for a in (x, wq, bq, wk, bk, wv, bv, wo, bo))
    nc = _get_nc()
    in_maps = make_in_maps(x, wq, bq, wk, bk, wv, bv, wo, bo)
    res = run_sharded(nc, in_maps)
    return gather(res.results)
